# revision 24
# baseline (speedup 1.0000x reference)
"""Cross-attention kernel for TRN2, 8 NeuronCores, data-parallel over points.

Math (identical to the reference, reassociated):
  qk[n]   = q[n] @ (Wq.T Wk) * s        (host, bf16)
  qp[n]   = q[n] @ Wq.T                 (host, bf16, shipped d-major)
  scores[n,w] = qk[n] . k[w,n]          (DVE mul + in-place adder tree)
  attn[n] = softmax_w(scores[n])        (ACT Exp, DVE recip, ACT scale)
  vmix[n] = sum_w attn[n,w] * v[w,n]    (DVE mul + adder tree, V innermost)
  y[n]    = gelu(vmix[n] @ (Wv.T Wo.T) + bo) + qp[n]
  out[c][8*i + j] = y[c*4096 + i]       (row replication on host)

Perf structure:
  - k/v fp8 e4m3 in HBM, upcast to bf16 during the SWDGE DMA (2x DVE mode).
  - Reductions as binary in-place adder trees (tensor_tensor runs 2x; the
    hardware tensor_reduce streams at 1x and is ~4x slower).
  - One DVE mul per 4-tile group (3 free-dim APs), trees batched per group.
  - Output projection runs d-major: stationary = MB quarters, moving = the
    transposed vmix batch, so gelu bias is a per-partition AP (no bias
    matmul) and the residual/store happen in d-major; host un-transposes.
  - Phase A (all scores) then phase B (all outputs): ACT loads the Exp and
    Gelu tables once each.
"""

import ml_dtypes
import numpy as np

import concourse.bass as bass
import concourse.mybir as mybir
import concourse.tile as tile
from concourse import bacc
from concourse.bass_utils import run_bass_kernel_spmd

N_CORES = 8
N_TOTAL = 32768
NC_PTS = N_TOTAL // N_CORES  # 4096 points per core
D = 256
V = 8
P = 128
G = 4  # tiles per group
N_TILES = NC_PTS // P  # 32
F32 = mybir.dt.float32
BF16 = mybir.dt.bfloat16
FP8 = mybir.dt.float8e4
NP_BF16 = ml_dtypes.bfloat16
NP_FP8 = ml_dtypes.float8_e4m3
AX = mybir.AxisListType
OP = mybir.AluOpType
AF = mybir.ActivationFunctionType


def _bcast(ap, axis_count, after_dims):
    """Insert a [0, axis_count] broadcast dim before the last `after_dims`
    dims of `ap`'s access pattern."""
    dims = list(ap.ap)
    pos = len(dims) - after_dims
    dims = dims[:pos] + [[0, axis_count]] + dims[pos:]
    return bass.AP(tensor=ap.tensor, offset=ap.offset, ap=dims)


def build_bass(n_tiles: int = N_TILES):
    nc = bacc.Bacc(
        "TRN2", target_bir_lowering=False, debug=False, num_devices=N_CORES
    )
    assert n_tiles % G == 0
    n_groups = n_tiles // G
    qk_d = nc.dram_tensor("qk", [P, n_tiles, D], BF16, kind="ExternalInput")
    # residual, d-major: qpT[p, h, t, n] = qp[t*128+n, h*128+p]
    qpT_d = nc.dram_tensor("qpT", [P, 2, n_tiles, P], BF16, kind="ExternalInput")
    k_d = nc.dram_tensor("k8", [P, n_tiles, V, D], FP8, kind="ExternalInput")
    v_d = nc.dram_tensor("v8", [P, n_tiles, D, V], FP8, kind="ExternalInput")
    mb_d = nc.dram_tensor("mb", [D, D], BF16, kind="ExternalInput")
    bo_d = nc.dram_tensor("bo2", [P, 2], F32, kind="ExternalInput")
    id_d = nc.dram_tensor("ident", [P, P], BF16, kind="ExternalInput")
    # d-major output; host un-transposes and replicates
    out_d = nc.dram_tensor("out", [P, 2, n_tiles, P], BF16, kind="ExternalOutput")

    with tile.TileContext(nc) as tc:
        with (
            tc.tile_pool(name="singles", bufs=1) as singles,
            tc.tile_pool(name="io", bufs=3) as io,
            tc.tile_pool(name="work", bufs=3) as work,
            tc.tile_pool(name="small", bufs=3) as small,
            tc.tile_pool(name="ps", bufs=2, space="PSUM") as ps,
        ):
            mb_t = singles.tile([P, 2, D], BF16)
            bo_t = singles.tile([P, 2], F32)
            id_t = singles.tile([P, P], BF16)
            attn_all = singles.tile([P, n_tiles, V], BF16)
            nc.sync.dma_start(
                out=mb_t, in_=mb_d.ap().rearrange("(h p) d -> p h d", p=P)
            )
            nc.sync.dma_start(out=bo_t, in_=bo_d.ap())
            nc.sync.dma_start(out=id_t, in_=id_d.ap())

            with nc.allow_low_precision(
                "bf16 stores tolerated (fp8 inputs dominate the error)"
            ):
                # ---- phase A: attention weights for all tiles ----
                # ramp-up: small first groups so the first mul isn't gated
                # on a full 1MB cast-DMA. The fp8->bf16 upcast alternates
                # between the serial SWDGE cast queue and the ACT engine so
                # neither becomes the per-group floor.
                plan = [1, 1, 2] + [G] * ((n_tiles - 4) // G)
                assert sum(plan) == n_tiles
                offs = [sum(plan[:i]) for i in range(len(plan))]
                # ramp groups prefetch up-front: HWDGE fp8 load + ACT upcast
                # beats the ~6us first-byte latency of the SWDGE cast queue,
                # and emitting all three casts before any Exp keeps the
                # in-order ACT queue from serializing cast(i+1) behind exp(i)
                ramp_kbs = {}
                for idx in range(3):
                    g0, g = offs[idx], plan[idx]
                    k8_t = io.tile([P, g, V, D], FP8, tag="f8")
                    nc.sync.dma_start(out=k8_t, in_=k_d.ap()[:, g0 : g0 + g])
                    kb = io.tile([P, g, V, D], BF16, tag="kv")
                    nc.scalar.copy(kb, k8_t)
                    ramp_kbs[idx] = kb

                # steady groups: half the tiles cast on the SWDGE queue, the
                # other half loaded fp8 via HWDGE and upcast on ACT, emitted
                # one group ahead so neither cast path paces the DVE
                def load_k_steady(idx):
                    g0, g = offs[idx], plan[idx]
                    h = g // 2
                    kb = io.tile(
                        [P, g, V, D], BF16, tag="kv", name=f"kbs_{idx}"
                    )
                    nc.gpsimd.dma_start(
                        out=kb[:, 0:h], in_=k_d.ap()[:, g0 : g0 + h]
                    )
                    k8_t = io.tile(
                        [P, g - h, V, D], FP8, tag="f8", name=f"k8s_{idx}"
                    )
                    nc.sync.dma_start(
                        out=k8_t, in_=k_d.ap()[:, g0 + h : g0 + g]
                    )
                    nc.scalar.copy(kb[:, h:g], k8_t)
                    ramp_kbs[idx] = kb

                load_k_steady(3)
                for idx, g in enumerate(plan):
                    g0 = offs[idx]
                    if idx + 1 > 3 and idx + 1 < len(plan):
                        load_k_steady(idx + 1)
                    kb = ramp_kbs.pop(idx)
                    qk_g = io.tile([P, g, D], BF16, tag="qq")
                    nc.sync.dma_start(out=qk_g, in_=qk_d.ap()[:, g0 : g0 + g])

                    prod = work.tile([P, g, V, D], BF16, tag="prod")
                    nc.vector.tensor_tensor(
                        prod, kb, _bcast(qk_g, V, 1), op=OP.mult
                    )
                    # sum over d: first tree level into scratch (half on the
                    # otherwise-idle GpSimd), then in place on DVE
                    tA = work.tile([P, g, V, D // 2], BF16, tag="l1")
                    nc.vector.tensor_tensor(
                        tA[:, :, 0 : V // 2],
                        prod[:, :, 0 : V // 2, 0 : D // 2],
                        prod[:, :, 0 : V // 2, D // 2 : D],
                        op=OP.add,
                    )
                    nc.gpsimd.tensor_tensor(
                        tA[:, :, V // 2 : V],
                        prod[:, :, V // 2 : V, 0 : D // 2],
                        prod[:, :, V // 2 : V, D // 2 : D],
                        op=OP.add,
                    )
                    sz = D // 4
                    while sz >= 2:
                        nc.vector.tensor_tensor(
                            tA[:, :, :, 0:sz],
                            tA[:, :, :, 0:sz],
                            tA[:, :, :, sz : 2 * sz],
                            op=OP.add,
                        )
                        sz //= 2
                    scores = small.tile([P, g, V], BF16, tag="scores")
                    nc.vector.tensor_tensor(
                        scores, tA[:, :, :, 0:1], tA[:, :, :, 1:2], op=OP.add
                    )
                    e_g = small.tile([P, g, V], BF16, tag="e")
                    nc.scalar.activation(e_g, scores, AF.Exp)
                    sm_g = small.tile([P, g], F32, tag="sm")
                    nc.vector.tensor_reduce(sm_g, e_g, axis=AX.X, op=OP.add)
                    rs_g = small.tile([P, g], F32, tag="rs")
                    nc.vector.reciprocal(rs_g, sm_g)
                    for t in range(g):
                        nc.scalar.mul(
                            attn_all[:, g0 + t], e_g[:, t], rs_g[:, t : t + 1]
                        )

                # ---- phase B: mix v, project (d-major), activate, store ----
                plan_b = [G] * n_groups
                offs_b = [sum(plan_b[:i]) for i in range(len(plan_b))]
                vbs = {}

                def load_v_steady(bi):
                    g0, g = offs_b[bi], plan_b[bi]
                    h = g // 2
                    vb = io.tile(
                        [P, g, D, V], BF16, tag="kv", name=f"vbs_{bi}"
                    )
                    nc.gpsimd.dma_start(
                        out=vb[:, 0:h], in_=v_d.ap()[:, g0 : g0 + h]
                    )
                    v8_t = io.tile(
                        [P, g - h, D, V], FP8, tag="f8", name=f"v8s_{bi}"
                    )
                    nc.sync.dma_start(
                        out=v8_t, in_=v_d.ap()[:, g0 + h : g0 + g]
                    )
                    nc.scalar.copy(vb[:, h:g], v8_t)
                    vbs[bi] = vb

                pending = None  # residual-add deferred by one group
                load_v_steady(0)
                for bi, g in enumerate(plan_b):
                    g0 = offs_b[bi]
                    if bi + 1 < len(plan_b):
                        load_v_steady(bi + 1)
                    vb = vbs.pop(bi)
                    qpT_g = io.tile([P, 2, g, P], BF16, tag="qq")
                    nc.sync.dma_start(
                        out=qpT_g, in_=qpT_d.ap()[:, :, g0 : g0 + g]
                    )

                    prod2 = work.tile([P, g, D, V], BF16, tag="prod")
                    nc.vector.tensor_tensor(
                        prod2,
                        vb,
                        _bcast(attn_all[:, g0 : g0 + g], D, 1),
                        op=OP.mult,
                    )
                    tB = work.tile([P, g, D, 4], BF16, tag="l1")
                    nc.vector.tensor_tensor(
                        tB[:, :, 0 : D // 2],
                        prod2[:, :, 0 : D // 2, 0:4],
                        prod2[:, :, 0 : D // 2, 4:8],
                        op=OP.add,
                    )
                    nc.gpsimd.tensor_tensor(
                        tB[:, :, D // 2 : D],
                        prod2[:, :, D // 2 : D, 0:4],
                        prod2[:, :, D // 2 : D, 4:8],
                        op=OP.add,
                    )
                    nc.vector.tensor_tensor(
                        tB[:, :, :, 0:2],
                        tB[:, :, :, 0:2],
                        tB[:, :, :, 2:4],
                        op=OP.add,
                    )
                    vmix = small.tile([P, g, D], BF16, tag="vmix")
                    nc.vector.tensor_tensor(
                        vmix, tB[:, :, :, 0:1], tB[:, :, :, 1:2], op=OP.add
                    )

                    # transpose vmix -> [din, n] halves, batched per group
                    vT_ps = ps.tile([P, 2, g, P], BF16, tag="vT")
                    for t in range(g):
                        nc.tensor.transpose(
                            vT_ps[:, 0, t], vmix[:, t, 0:P], id_t
                        )
                        nc.tensor.transpose(
                            vT_ps[:, 1, t], vmix[:, t, P:D], id_t
                        )
                    vTg = small.tile([P, 2, g * P], BF16, tag="vTg")
                    nc.scalar.copy(vTg, vT_ps)

                    # ylinT[dout_h] = sum_hin MB[hin, dout_h].T @ vT[hin]
                    ylin_ps = ps.tile([P, 2, g * P], F32, tag="ylin")
                    for ho in range(2):
                        nc.tensor.matmul(
                            ylin_ps[:, ho],
                            mb_t[:, 0, ho * P : (ho + 1) * P],
                            vTg[:, 0],
                            start=True,
                            stop=False,
                        )
                        nc.tensor.matmul(
                            ylin_ps[:, ho],
                            mb_t[:, 1, ho * P : (ho + 1) * P],
                            vTg[:, 1],
                            start=False,
                            stop=True,
                        )
                    if pending is not None:
                        p_gl, p_qp, p_g0, p_g = pending
                        yo = small.tile([P, 2, p_g * P], BF16, tag="yo")
                        nc.vector.tensor_tensor(yo, p_gl, p_qp, op=OP.add)
                        nc.scalar.dma_start(
                            out=out_d.ap()[:, :, p_g0 : p_g0 + p_g], in_=yo
                        )
                    gl = small.tile([P, 2, g * P], BF16, tag="gl")
                    for ho in range(2):
                        nc.scalar.activation(
                            gl[:, ho],
                            ylin_ps[:, ho],
                            AF.Gelu,
                            bias=bo_t[:, ho : ho + 1],
                        )
                    pending = (gl, qpT_g, g0, g)
                p_gl, p_qp, p_g0, p_g = pending
                yo = small.tile([P, 2, p_g * P], BF16, tag="yo")
                nc.vector.tensor_tensor(yo, p_gl, p_qp, op=OP.add)
                nc.scalar.dma_start(
                    out=out_d.ap()[:, :, p_g0 : p_g0 + p_g], in_=yo
                )

    nc.compile()
    return nc


_NC_CACHE = {}


def _get_nc(n_tiles: int = N_TILES):
    if n_tiles not in _NC_CACHE:
        _NC_CACHE[n_tiles] = build_bass(n_tiles)
    return _NC_CACHE[n_tiles]


def _host_prep(Wq, Wk, Wv, Wo, bo):
    Wq = np.asarray(Wq, dtype=np.float32)
    Wk = np.asarray(Wk, dtype=np.float32)
    Wv = np.asarray(Wv, dtype=np.float32)
    Wo = np.asarray(Wo, dtype=np.float32)
    bo = np.asarray(bo, dtype=np.float32)
    scale = np.float32(1.0) / np.sqrt(np.float32(D))
    ma = (Wq.T @ Wk) * scale
    mb = np.ascontiguousarray(Wv.T @ Wo.T).astype(NP_BF16)
    bo2 = np.ascontiguousarray(bo.reshape(2, P).T, dtype=np.float32)
    ident = np.eye(P, dtype=NP_BF16)
    return ma, mb, bo2, ident


def _tile_pm(x, last_dims):
    """[NC_PTS, *last] -> [P, N_TILES, *last] partition-major."""
    return np.ascontiguousarray(
        x.reshape(N_TILES, P, *last_dims).transpose(
            1, 0, *range(2, 2 + len(last_dims))
        )
    )


def make_in_maps(q, k, v, Wq, Wk, Wv, Wo, bo):
    q = np.asarray(q, dtype=np.float32)
    k = np.asarray(k, dtype=np.float32)
    v = np.asarray(v, dtype=np.float32)
    ma, mb, bo2, ident = _host_prep(Wq, Wk, Wv, Wo, bo)
    Wq32 = np.asarray(Wq, dtype=np.float32)
    qk_full = (q[0] @ ma).astype(NP_BF16)  # [N, D]
    qp_full = (q[0] @ Wq32.T).astype(NP_BF16)
    in_maps = []
    for c in range(N_CORES):
        sl = slice(c * NC_PTS, (c + 1) * NC_PTS)
        qk_c = _tile_pm(qk_full[sl], (D,))
        # d-major residual: [t*128+n, h*128+p] -> [p, h, t, n]
        qpT_c = np.ascontiguousarray(
            qp_full[sl].reshape(N_TILES, P, 2, P).transpose(3, 2, 0, 1)
        )
        k_c = _tile_pm(k[:, sl].transpose(1, 0, 2).astype(NP_FP8), (V, D))
        v_c = _tile_pm(v[:, sl].transpose(1, 2, 0).astype(NP_FP8), (D, V))
        in_maps.append(
            {
                "qk": qk_c,
                "qpT": qpT_c,
                "k8": k_c,
                "v8": v_c,
                "mb": mb,
                "bo2": bo2,
                "ident": ident,
            }
        )
    return in_maps


def gather_out(results):
    """[P, 2, N_TILES, P] bf16 d-major per core -> [8, 32768, 256] f32."""
    out = np.empty((N_CORES, N_TOTAL, D), dtype=np.float32)
    for c in range(N_CORES):
        y = (
            results[c]["out"]
            .transpose(2, 3, 1, 0)  # [t, n, h, p]
            .reshape(NC_PTS, D)
            .astype(np.float32)
        )
        out[c] = np.repeat(y, V, axis=0)
    return out


def kernel(q, k, v, Wq, Wk, Wv, Wo, bo):
    nc = _get_nc()
    in_maps = make_in_maps(q, k, v, Wq, Wk, Wv, Wo, bo)
    res = run_bass_kernel_spmd(nc, in_maps, core_ids=list(range(N_CORES)))
    return gather_out(res.results)


# revision 25
# speedup vs baseline: 1.2367x; 1.2367x over previous
"""Cross-attention kernel for TRN2, 8 NeuronCores, data-parallel over points.

Math (identical to the reference, reassociated):
  qk[n]   = q[n] @ (Wq.T Wk) * s        (host, bf16)
  qp[n]   = q[n] @ Wq.T                 (host, bf16, shipped d-major)
  scores[n,w] = qk[n] . k[w,n]          (DVE mul + in-place adder tree)
  attn[n] = softmax_w(scores[n])        (ACT Exp, DVE recip, ACT scale)
  vmix[n] = sum_w attn[n,w] * v[w,n]    (DVE mul + adder tree, V innermost)
  y[n]    = gelu(vmix[n] @ (Wv.T Wo.T) + bo) + qp[n]
  out[c][8*i + j] = y[c*4096 + i]       (row replication on host)

Perf structure:
  - k/v fp8 e4m3 in HBM, upcast to bf16 during the SWDGE DMA (2x DVE mode).
  - Reductions as binary in-place adder trees (tensor_tensor runs 2x; the
    hardware tensor_reduce streams at 1x and is ~4x slower).
  - One DVE mul per 4-tile group (3 free-dim APs), trees batched per group.
  - Output projection runs d-major: stationary = MB quarters, moving = the
    transposed vmix batch, so gelu bias is a per-partition AP (no bias
    matmul) and the residual/store happen in d-major; host un-transposes.
  - Phase A (all scores) then phase B (all outputs): ACT loads the Exp and
    Gelu tables once each.
"""

import ml_dtypes
import numpy as np

import concourse.bass as bass
import concourse.mybir as mybir
import concourse.tile as tile
from concourse import bacc
from concourse.bass_utils import run_bass_kernel_spmd

N_CORES = 8
N_TOTAL = 32768
NC_PTS = N_TOTAL // N_CORES  # 4096 points per core
D = 256
V = 8
P = 128
G = 4  # tiles per group
N_TILES = NC_PTS // P  # 32
F32 = mybir.dt.float32
BF16 = mybir.dt.bfloat16
FP8 = mybir.dt.float8e4
NP_BF16 = ml_dtypes.bfloat16
NP_FP8 = ml_dtypes.float8_e4m3
AX = mybir.AxisListType
OP = mybir.AluOpType
AF = mybir.ActivationFunctionType


def _bcast(ap, axis_count, after_dims):
    """Insert a [0, axis_count] broadcast dim before the last `after_dims`
    dims of `ap`'s access pattern."""
    dims = list(ap.ap)
    pos = len(dims) - after_dims
    dims = dims[:pos] + [[0, axis_count]] + dims[pos:]
    return bass.AP(tensor=ap.tensor, offset=ap.offset, ap=dims)


def build_bass(n_tiles: int = N_TILES):
    nc = bacc.Bacc(
        "TRN2", target_bir_lowering=False, debug=False, num_devices=N_CORES
    )
    assert n_tiles % G == 0
    n_groups = n_tiles // G
    qk_d = nc.dram_tensor("qk", [P, n_tiles, D], BF16, kind="ExternalInput")
    # residual, d-major: qpT[p, h, t, n] = qp[t*128+n, h*128+p]
    qpT_d = nc.dram_tensor("qpT", [P, 2, n_tiles, P], BF16, kind="ExternalInput")
    k_d = nc.dram_tensor("k8", [P, n_tiles, V, D], FP8, kind="ExternalInput")
    v_d = nc.dram_tensor("v8", [P, n_tiles, D, V], FP8, kind="ExternalInput")
    mb_d = nc.dram_tensor("mb", [D, D], BF16, kind="ExternalInput")
    bo_d = nc.dram_tensor("bo2", [P, 2], F32, kind="ExternalInput")
    id_d = nc.dram_tensor("ident", [P, P], BF16, kind="ExternalInput")
    # d-major output; host un-transposes and replicates
    out_d = nc.dram_tensor("out", [P, 2, n_tiles, P], BF16, kind="ExternalOutput")

    with tile.TileContext(nc) as tc:
        with (
            tc.tile_pool(name="singles", bufs=1) as singles,
            tc.tile_pool(name="io", bufs=3) as io,
            tc.tile_pool(name="work", bufs=3) as work,
            tc.tile_pool(name="small", bufs=3) as small,
            tc.tile_pool(name="ps", bufs=2, space="PSUM") as ps,
        ):
            mb_t = singles.tile([P, 2, D], BF16)
            bo_t = singles.tile([P, 2], F32)
            id_t = singles.tile([P, P], BF16)
            attn_all = singles.tile([P, n_tiles, V], BF16)
            nc.sync.dma_start(
                out=mb_t, in_=mb_d.ap().rearrange("(h p) d -> p h d", p=P)
            )
            nc.sync.dma_start(out=bo_t, in_=bo_d.ap())
            nc.sync.dma_start(out=id_t, in_=id_d.ap())

            with nc.allow_low_precision(
                "bf16 stores tolerated (fp8 inputs dominate the error)"
            ):
                # ---- phase A: attention weights for all tiles ----
                # ramp-up: small first groups so the first mul isn't gated
                # on a full 1MB cast-DMA. The fp8->bf16 upcast alternates
                # between the serial SWDGE cast queue and the ACT engine so
                # neither becomes the per-group floor.
                plan = [1, 1, 2] + [G] * ((n_tiles - 4) // G)
                assert sum(plan) == n_tiles
                offs = [sum(plan[:i]) for i in range(len(plan))]
                for idx, g in enumerate(plan):
                    g0 = offs[idx]
                    kb = io.tile([P, g, V, D], BF16, tag="kv")
                    if idx < 3:
                        # ramp groups: HWDGE fp8 load + ACT upcast beats the
                        # ~6us first-byte latency of the SWDGE cast queue
                        k8_t = io.tile([P, g, V, D], FP8, tag="f8")
                        nc.sync.dma_start(
                            out=k8_t, in_=k_d.ap()[:, g0 : g0 + g]
                        )
                        nc.scalar.copy(kb, k8_t)
                    else:
                        nc.gpsimd.dma_start(
                            out=kb, in_=k_d.ap()[:, g0 : g0 + g]
                        )
                    qk_g = io.tile([P, g, D], BF16, tag="qq")
                    nc.sync.dma_start(out=qk_g, in_=qk_d.ap()[:, g0 : g0 + g])

                    prod = work.tile([P, g, V, D], BF16, tag="prod")
                    nc.vector.tensor_tensor(
                        prod, kb, _bcast(qk_g, V, 1), op=OP.mult
                    )
                    # sum over d: first tree level into scratch, then in place
                    tA = work.tile([P, g, V, D // 2], BF16, tag="l1")
                    nc.vector.tensor_tensor(
                        tA,
                        prod[:, :, :, 0 : D // 2],
                        prod[:, :, :, D // 2 : D],
                        op=OP.add,
                    )
                    sz = D // 4
                    while sz >= 2:
                        nc.vector.tensor_tensor(
                            tA[:, :, :, 0:sz],
                            tA[:, :, :, 0:sz],
                            tA[:, :, :, sz : 2 * sz],
                            op=OP.add,
                        )
                        sz //= 2
                    scores = small.tile([P, g, V], BF16, tag="scores")
                    nc.vector.tensor_tensor(
                        scores, tA[:, :, :, 0:1], tA[:, :, :, 1:2], op=OP.add
                    )
                    e_g = small.tile([P, g, V], BF16, tag="e")
                    nc.scalar.activation(e_g, scores, AF.Exp)
                    sm_g = small.tile([P, g], F32, tag="sm")
                    nc.vector.tensor_reduce(sm_g, e_g, axis=AX.X, op=OP.add)
                    rs_g = small.tile([P, g], F32, tag="rs")
                    nc.vector.reciprocal(rs_g, sm_g)
                    for t in range(g):
                        nc.scalar.mul(
                            attn_all[:, g0 + t], e_g[:, t], rs_g[:, t : t + 1]
                        )

                # ---- phase B: mix v, project (d-major), activate, store ----
                plan_b = [G] * n_groups
                offs_b = [sum(plan_b[:i]) for i in range(len(plan_b))]
                pending = None  # residual-add deferred by one group
                for bi, g in enumerate(plan_b):
                    g0 = offs_b[bi]
                    vb = io.tile([P, g, D, V], BF16, tag="kv")
                    nc.gpsimd.dma_start(out=vb, in_=v_d.ap()[:, g0 : g0 + g])
                    qpT_g = io.tile([P, 2, g, P], BF16, tag="qq")
                    nc.sync.dma_start(
                        out=qpT_g, in_=qpT_d.ap()[:, :, g0 : g0 + g]
                    )

                    prod2 = work.tile([P, g, D, V], BF16, tag="prod")
                    nc.vector.tensor_tensor(
                        prod2,
                        vb,
                        _bcast(attn_all[:, g0 : g0 + g], D, 1),
                        op=OP.mult,
                    )
                    tB = work.tile([P, g, D, 4], BF16, tag="l1")
                    nc.vector.tensor_tensor(
                        tB, prod2[:, :, :, 0:4], prod2[:, :, :, 4:8], op=OP.add
                    )
                    nc.vector.tensor_tensor(
                        tB[:, :, :, 0:2],
                        tB[:, :, :, 0:2],
                        tB[:, :, :, 2:4],
                        op=OP.add,
                    )
                    vmix = small.tile([P, g, D], BF16, tag="vmix")
                    nc.vector.tensor_tensor(
                        vmix, tB[:, :, :, 0:1], tB[:, :, :, 1:2], op=OP.add
                    )

                    # transpose vmix -> [din, n] halves, batched per group
                    vT_ps = ps.tile([P, 2, g, P], BF16, tag="vT")
                    for t in range(g):
                        nc.tensor.transpose(
                            vT_ps[:, 0, t], vmix[:, t, 0:P], id_t
                        )
                        nc.tensor.transpose(
                            vT_ps[:, 1, t], vmix[:, t, P:D], id_t
                        )
                    vTg = small.tile([P, 2, g * P], BF16, tag="vTg")
                    nc.scalar.copy(vTg, vT_ps)

                    # ylinT[dout_h] = sum_hin MB[hin, dout_h].T @ vT[hin]
                    ylin_ps = ps.tile([P, 2, g * P], F32, tag="ylin")
                    for ho in range(2):
                        nc.tensor.matmul(
                            ylin_ps[:, ho],
                            mb_t[:, 0, ho * P : (ho + 1) * P],
                            vTg[:, 0],
                            start=True,
                            stop=False,
                        )
                        nc.tensor.matmul(
                            ylin_ps[:, ho],
                            mb_t[:, 1, ho * P : (ho + 1) * P],
                            vTg[:, 1],
                            start=False,
                            stop=True,
                        )
                    if pending is not None:
                        p_gl, p_qp, p_g0, p_g = pending
                        yo = small.tile([P, 2, p_g * P], BF16, tag="yo")
                        nc.vector.tensor_tensor(yo, p_gl, p_qp, op=OP.add)
                        nc.scalar.dma_start(
                            out=out_d.ap()[:, :, p_g0 : p_g0 + p_g], in_=yo
                        )
                    gl = small.tile([P, 2, g * P], BF16, tag="gl")
                    for ho in range(2):
                        nc.scalar.activation(
                            gl[:, ho],
                            ylin_ps[:, ho],
                            AF.Gelu,
                            bias=bo_t[:, ho : ho + 1],
                        )
                    pending = (gl, qpT_g, g0, g)
                p_gl, p_qp, p_g0, p_g = pending
                yo = small.tile([P, 2, p_g * P], BF16, tag="yo")
                nc.vector.tensor_tensor(yo, p_gl, p_qp, op=OP.add)
                nc.scalar.dma_start(
                    out=out_d.ap()[:, :, p_g0 : p_g0 + p_g], in_=yo
                )

    nc.compile()
    return nc


_NC_CACHE = {}


def _get_nc(n_tiles: int = N_TILES):
    if n_tiles not in _NC_CACHE:
        _NC_CACHE[n_tiles] = build_bass(n_tiles)
    return _NC_CACHE[n_tiles]


def _host_prep(Wq, Wk, Wv, Wo, bo):
    Wq = np.asarray(Wq, dtype=np.float32)
    Wk = np.asarray(Wk, dtype=np.float32)
    Wv = np.asarray(Wv, dtype=np.float32)
    Wo = np.asarray(Wo, dtype=np.float32)
    bo = np.asarray(bo, dtype=np.float32)
    scale = np.float32(1.0) / np.sqrt(np.float32(D))
    ma = (Wq.T @ Wk) * scale
    mb = np.ascontiguousarray(Wv.T @ Wo.T).astype(NP_BF16)
    bo2 = np.ascontiguousarray(bo.reshape(2, P).T, dtype=np.float32)
    ident = np.eye(P, dtype=NP_BF16)
    return ma, mb, bo2, ident


def _tile_pm(x, last_dims):
    """[NC_PTS, *last] -> [P, N_TILES, *last] partition-major."""
    return np.ascontiguousarray(
        x.reshape(N_TILES, P, *last_dims).transpose(
            1, 0, *range(2, 2 + len(last_dims))
        )
    )


def make_in_maps(q, k, v, Wq, Wk, Wv, Wo, bo):
    q = np.asarray(q, dtype=np.float32)
    k = np.asarray(k, dtype=np.float32)
    v = np.asarray(v, dtype=np.float32)
    ma, mb, bo2, ident = _host_prep(Wq, Wk, Wv, Wo, bo)
    Wq32 = np.asarray(Wq, dtype=np.float32)
    qk_full = (q[0] @ ma).astype(NP_BF16)  # [N, D]
    qp_full = (q[0] @ Wq32.T).astype(NP_BF16)
    in_maps = []
    for c in range(N_CORES):
        sl = slice(c * NC_PTS, (c + 1) * NC_PTS)
        qk_c = _tile_pm(qk_full[sl], (D,))
        # d-major residual: [t*128+n, h*128+p] -> [p, h, t, n]
        qpT_c = np.ascontiguousarray(
            qp_full[sl].reshape(N_TILES, P, 2, P).transpose(3, 2, 0, 1)
        )
        k_c = _tile_pm(k[:, sl].transpose(1, 0, 2).astype(NP_FP8), (V, D))
        v_c = _tile_pm(v[:, sl].transpose(1, 2, 0).astype(NP_FP8), (D, V))
        in_maps.append(
            {
                "qk": qk_c,
                "qpT": qpT_c,
                "k8": k_c,
                "v8": v_c,
                "mb": mb,
                "bo2": bo2,
                "ident": ident,
            }
        )
    return in_maps


def gather_out(results):
    """[P, 2, N_TILES, P] bf16 d-major per core -> [8, 32768, 256] f32."""
    out = np.empty((N_CORES, N_TOTAL, D), dtype=np.float32)
    for c in range(N_CORES):
        y = (
            results[c]["out"]
            .transpose(2, 3, 1, 0)  # [t, n, h, p]
            .reshape(NC_PTS, D)
            .astype(np.float32)
        )
        out[c] = np.repeat(y, V, axis=0)
    return out


def kernel(q, k, v, Wq, Wk, Wv, Wo, bo):
    nc = _get_nc()
    in_maps = make_in_maps(q, k, v, Wq, Wk, Wv, Wo, bo)
    res = run_bass_kernel_spmd(nc, in_maps, core_ids=list(range(N_CORES)))
    return gather_out(res.results)
